# revision 2
# baseline (speedup 1.0000x reference)
"""GCN (3-layer GCNConv + mean-pool + MLP head) Trainium2 Bass kernel, 8 NeuronCores.

v2: bulk hardware gather via gpsimd.dma_gather (SWDGE descriptor gather,
~0.34ns/descriptor desc-gen) replaces the per-column indirect DMA of the
baseline (~15us/instruction software descriptor generation).

Strategy:
  - Destination nodes partitioned into 8 contiguous blocks (one per core),
    SHARD=12544 padded rows each; node tables live in DRAM padded to 128
    bf16 elements per row (256B, the dma_gather granularity).
  - Self-loops folded into the edge list as ordinary edges (weight 1/deg).
  - dma_gather indices are int16, so the 100352-row table is addressed in
    4 quarters of 25088 rows; edges are packed into 128-edge columns
    grouped by (chunk of 7 dst windows, src quarter, dst window) with a
    fixed CC columns per (window, quarter) group -> fully static layout,
    identical across cores (SPMD-safe).
  - One dma_gather per (chunk, quarter) fetches CW*CC columns of message
    rows; messages are scaled by the GCN edge norm and scattered into a
    per-window PSUM accumulator with one-hot matmuls, then transformed
    (aggregate-then-transform), relu'd, and written to the padded shard.
  - Layer boundary: two AllGather segments (49 windows each, aligned to
    chunk boundaries) so the seg-0 collective overlaps second-half compute
    and the next layer's quarter-0/1 gathers overlap the seg-1 collective.
  - Layer 3 output is mean-pooled per graph (one-hot matmul into a PSUM
    accumulator), AllReduced, and the tiny FC head runs replicated.
"""

import os
import sys

import numpy as np
import ml_dtypes

for _p in ("/opt/trn_rl_repo", "/root/.axon_site/_ro/trn_rl_repo"):
    if os.path.isdir(_p) and _p not in sys.path:
        sys.path.insert(0, _p)

bf16 = ml_dtypes.bfloat16
P = 128

N = 100000
G = 128
SHARD = 12544
NW = 98           # dst windows per core
CW = 7            # windows per chunk
NCHUNK = NW // CW
SEG = SHARD // 2  # AllGather segment rows (= 49 windows)
NPAD = 8 * SHARD  # 100352
QR = NPAD // 4    # quarter rows: 25088 (int16-addressable)
FS = (40, 40, 80, 160)
HID = 128
NCORES = 8
ELEM = 128        # padded table row elements (256B)
SCRATCH = 16384   # SWDGE descriptor scratch (ring capacity is runtime-fixed)
MAXCOLS = 8       # max 128-edge columns per dma_gather call (ring ~80 descs)


# ---------------------------------------------------------------- host prep

def _remap(v):
    c, r = v // SHARD, v % SHARD
    s = r // SEG
    return s * (NCORES * SEG) + c * SEG + (r - s * SEG)


def call_ranges(cc):
    """Gather-call column ranges: each (chunk, quarter) group of CW*cc
    columns split into sub-calls of <= MAXCOLS columns (SWDGE ring limit)."""
    ccall = CW * cc
    nsplit = -(-ccall // MAXCOLS)
    out = []
    for kc in range(NCHUNK):
        for kq in range(4):
            a = ((kc * 4 + kq) * CW) * cc
            for s in range(nsplit):
                lo = a + s * ccall // nsplit
                hi = a + (s + 1) * ccall // nsplit
                if hi > lo:
                    out.append((lo, hi))
    return out


def build_host_data(inp, cc):
    """Pack edges (incl. self-loops) into the fixed grid: group
    (chunk, quarter, window-in-chunk) owns `cc` 128-edge columns."""
    src = np.asarray(inp["edge_index"][0]).astype(np.int64).ravel()
    dst = np.asarray(inp["edge_index"][1]).astype(np.int64).ravel()
    batch = np.asarray(inp["batch"]).astype(np.int64).ravel()

    deg = (np.bincount(dst, minlength=N) + 1).astype(np.float32)
    dis = 1.0 / np.sqrt(deg)

    loop = np.arange(N, dtype=np.int64)
    srcA = np.concatenate([src, loop])
    dstA = np.concatenate([dst, loop])
    norm = (dis[srcA] * dis[dstA]).astype(np.float32)

    rsrc = _remap(srcA)
    q = rsrc // QR
    qi = (rsrc % QR).astype(np.int16)
    core = dstA // SHARD
    w = (dstA % SHARD) // P
    dloc = (dstA % SHARD) % P

    COLS = NW * 4 * cc  # group (w, q) -> columns gbase(w, q) + [0, cc)

    def gbase(w_, q_):
        kc, wl = w_ // CW, w_ % CW
        return ((kc * 4 + q_) * CW + wl) * cc

    order = np.lexsort((q, w, core))
    qA, qiA, coreA, wA, dlocA, normA = (
        a[order] for a in (q, qi, core, w, dloc, norm))

    cores = []
    for c in range(NCORES):
        lo, hi = np.searchsorted(coreA, [c, c + 1])
        cq, cqi, cw_, cdl, cnm = (
            a[lo:hi] for a in (qA, qiA, wA, dlocA, normA))
        gkey = cw_ * 4 + cq  # sorted (w asc, q asc) by the lexsort
        ukeys, counts = np.unique(gkey, return_counts=True)
        assert counts.max() <= cc * P, f"group overflow {counts.max()} > {cc * P}"
        estart = np.concatenate([[0], np.cumsum(counts)])
        jin = np.arange(len(cq)) - np.repeat(estart[:-1], counts)
        gb = np.array([gbase(k // 4, k % 4) for k in ukeys])
        col = np.repeat(gb, counts) + jin // P
        pp = jin % P

        eidx = np.zeros((P, COLS), np.int16)
        edl = np.full((P, COLS), -1.0, bf16)
        enrm = np.zeros((P, COLS), bf16)
        eidx[pp, col] = cqi
        edl[pp, col] = cdl.astype(bf16)
        enrm[pp, col] = cnm.astype(bf16)

        # wrapped idx per gather call (sub-ranges of a chunk-quarter group):
        # idx k of a call -> wrapped [partition k%16 (tiled x8), col k//16]
        widx = np.zeros((P, COLS * 8), np.int16)
        for a, b in call_ranges(cc):
            arr = eidx[:, a:b].T.ravel()
            wrap = arr.reshape(-1, 16).T
            widx[:, a * 8:b * 8] = np.tile(wrap, (8, 1))

        nid = np.arange(SHARD) + c * SHARD
        gl = np.where(nid < N, batch[np.minimum(nid, N - 1)], -1).astype(np.float32)
        gloc = np.ascontiguousarray(gl.reshape(NW, P).T).astype(bf16)

        cores.append(dict(eidxw=widx, edl=edl, enrm=enrm, gloc=gloc))

    xtp = np.zeros((NPAD, ELEM), bf16)
    xtp[_remap(np.arange(N)), :FS[0]] = np.asarray(inp["x"]).astype(bf16)

    cnt = np.bincount(batch, minlength=G).astype(np.float32)
    invc = np.zeros((P, 1), np.float32)
    invc[:G, 0] = 1.0 / np.maximum(cnt, 1.0)

    def a2(x, dt):
        return np.ascontiguousarray(np.asarray(x), dtype=dt)

    wts = dict(
        w1a=np.concatenate([a2(inp["W1"], bf16), a2(inp["b1"], bf16)[None]], 0),
        w2a=np.concatenate([a2(inp["W2"], bf16), a2(inp["b2"], bf16)[None]], 0),
        w3a=np.concatenate([a2(inp["W3"], bf16), a2(inp["b3"], bf16)[None]], 0),
        fw1=a2(inp["fW1"], bf16),
        fb1c=a2(inp["fb1"], np.float32).reshape(-1, 1),
        fw2=a2(inp["fW2"], bf16),
        invc=invc,
        iob=np.arange(P, dtype=np.float32)[None].repeat(P, 0).astype(bf16),
        idn=np.eye(P, dtype=bf16),
    )
    fb2 = float(np.asarray(inp["fb2"]).ravel()[0])
    return cores, xtp, wts, fb2


def derive_cc(inp):
    """cc = max columns any (dst-window, src-quarter) group needs."""
    src = np.asarray(inp["edge_index"][0]).astype(np.int64).ravel()
    dst = np.asarray(inp["edge_index"][1]).astype(np.int64).ravel()
    loop = np.arange(N, dtype=np.int64)
    srcA = np.concatenate([src, loop])
    dstA = np.concatenate([dst, loop])
    q = _remap(srcA) // QR
    gw = dstA // P  # global window id (SHARD % 128 == 0)
    cnt = np.bincount(gw * 4 + q, minlength=(NPAD // P) * 4)
    return max(1, int(-(-cnt.max() // P)))


# ---------------------------------------------------------------- bass build

def build_bass(cc, fb2, reps=1):
    import concourse.bacc as bacc
    import concourse.mybir as mybir
    import concourse.tile as tile

    dt = mybir.dt
    AF = mybir.ActivationFunctionType
    OP = mybir.AluOpType
    F0, F1, F2, F3 = FS
    FMAX = max(F0, F1, F2)
    COLS = NW * 4 * cc
    CCALL = CW * cc

    nc = bacc.Bacc("TRN2", target_bir_lowering=False, debug=False,
                   enable_asserts=False, num_devices=NCORES,
                   num_swdge_queues=4, dynamic_dma_scratch_size=SCRATCH)

    grp_calls = {}
    for lo, hi in call_ranges(cc):
        g = lo // (CW * cc)
        grp_calls.setdefault((g // 4, g % 4), []).append((lo, hi))

    xtp_d = nc.dram_tensor("xtp", [NPAD, ELEM], dt.bfloat16, kind="ExternalInput")
    widx_d = nc.dram_tensor("eidxw", [P, COLS * 8], dt.int16, kind="ExternalInput")
    edl_d = nc.dram_tensor("edl", [P, COLS], dt.bfloat16, kind="ExternalInput")
    enrm_d = nc.dram_tensor("enrm", [P, COLS], dt.bfloat16, kind="ExternalInput")
    gloc_d = nc.dram_tensor("gloc", [P, NW], dt.bfloat16, kind="ExternalInput")
    w1a_d = nc.dram_tensor("w1a", [F0 + 1, F1], dt.bfloat16, kind="ExternalInput")
    w2a_d = nc.dram_tensor("w2a", [F1 + 1, F2], dt.bfloat16, kind="ExternalInput")
    w3a_d = nc.dram_tensor("w3a", [F2 + 1, F3], dt.bfloat16, kind="ExternalInput")
    fw1_d = nc.dram_tensor("fw1", [F3, HID], dt.bfloat16, kind="ExternalInput")
    fb1_d = nc.dram_tensor("fb1c", [HID, 1], dt.float32, kind="ExternalInput")
    fw2_d = nc.dram_tensor("fw2", [HID, 1], dt.bfloat16, kind="ExternalInput")
    invc_d = nc.dram_tensor("invc", [P, 1], dt.float32, kind="ExternalInput")
    iob_d = nc.dram_tensor("iob", [P, P], dt.bfloat16, kind="ExternalInput")
    idn_d = nc.dram_tensor("idn", [P, P], dt.bfloat16, kind="ExternalInput")
    out_d = nc.dram_tensor("out", [1, P], dt.float32, kind="ExternalOutput")

    rg = [list(range(NCORES))]
    HSEG = NCORES * SEG

    with tile.TileContext(nc) as tc:
        with (
            tc.tile_pool(name="res", bufs=1) as res,
            tc.tile_pool(name="msgs", bufs=8) as msgsp,
            tc.tile_pool(name="sp", bufs=3) as sp,
            tc.tile_pool(name="work", bufs=3) as work,
            tc.tile_pool(name="pa_ps", bufs=2, space="PSUM") as pa_ps,
            tc.tile_pool(name="p2_ps", bufs=2, space="PSUM") as p2_ps,
            tc.tile_pool(name="pool_ps", bufs=1, space="PSUM") as pool_ps,
            tc.tile_pool(name="head_ps", bufs=1, space="PSUM") as head_ps,
            tc.tile_pool(name="dram", bufs=1, space="DRAM") as dram,
        ):
            # ---- persistent SBUF state
            widx = res.tile([P, COLS * 8], dt.int16)
            edl = res.tile([P, COLS], dt.bfloat16)
            enrm = res.tile([P, COLS], dt.bfloat16)
            gloc = res.tile([P, NW], dt.bfloat16)
            w1a = res.tile([F0 + 1, F1], dt.bfloat16)
            w2a = res.tile([F1 + 1, F2], dt.bfloat16)
            w3a = res.tile([F2 + 1, F3], dt.bfloat16)
            fw1a = res.tile([F3 // 2, HID], dt.bfloat16)
            fw1b = res.tile([F3 // 2, HID], dt.bfloat16)
            fb1c = res.tile([HID, 1], dt.float32)
            fw2 = res.tile([HID, 1], dt.bfloat16)
            invc = res.tile([P, 1], dt.float32)
            b1r = res.tile([1, F1], dt.bfloat16)
            b2r = res.tile([1, F2], dt.bfloat16)
            b3r = res.tile([1, F3], dt.bfloat16)
            iota_b = res.tile([P, P], dt.bfloat16)
            ident = res.tile([P, P], dt.bfloat16)
            ones1 = res.tile([1, P], dt.bfloat16)
            for sb, dr in ((widx, widx_d), (edl, edl_d), (enrm, enrm_d),
                           (gloc, gloc_d), (w1a, w1a_d), (w2a, w2a_d),
                           (w3a, w3a_d), (fb1c, fb1_d), (fw2, fw2_d),
                           (invc, invc_d), (iota_b, iob_d), (ident, idn_d)):
                nc.sync.dma_start(out=sb[:], in_=dr[:])
            nc.sync.dma_start(out=b1r[:], in_=w1a_d[F0:F0 + 1, :])
            nc.sync.dma_start(out=b2r[:], in_=w2a_d[F1:F1 + 1, :])
            nc.sync.dma_start(out=b3r[:], in_=w3a_d[F2:F2 + 1, :])
            nc.sync.dma_start(out=fw1a[:], in_=fw1_d[0:F3 // 2, :])
            nc.sync.dma_start(out=fw1b[:], in_=fw1_d[F3 // 2:, :])
            nc.vector.memset(ones1[:], 1.0)

            # ---- DRAM tables (padded 256B rows) / shard bounce buffers
            h1s = dram.tile([SHARD, ELEM], dt.bfloat16)
            h2s = dram.tile([SHARD, ELEM], dt.bfloat16)
            h1t = dram.tile([NPAD, ELEM], dt.bfloat16)
            h2t = dram.tile([NPAD, ELEM], dt.bfloat16)
            pool_pt = dram.tile([P, F3], dt.float32)
            pool_rd = dram.tile([P, F3], dt.float32)

            pool_acc = pool_ps.tile([P, F3], dt.float32)
            qrot = [0]

            def layer(tbl, F_in, F_out, waug, brow, shard_out):
                last = F_in == F2

                def do_chunk(kc):
                    mts = []
                    for kq in range(4):
                        a = ((kc * 4 + kq) * CW) * cc
                        mt = msgsp.tile([P, CCALL, ELEM], dt.bfloat16, tag="mt")
                        for lo, hi in grp_calls[kc, kq]:
                            w_ = hi - lo
                            nc.gpsimd.dma_gather(
                                mt[:, lo - a:hi - a, :],
                                tbl[kq * QR:(kq + 1) * QR, :],
                                widx[:, lo * 8:hi * 8],
                                w_ * P, w_ * P, ELEM,
                                queue_num=qrot[0] % 4)
                            qrot[0] += 1
                        nc.vector.tensor_tensor(
                            out=mt[:, :, 0:F_in], in0=mt[:, :, 0:F_in],
                            in1=enrm[:, a:a + CCALL, None].broadcast_to(
                                [P, CCALL, F_in]),
                            op=OP.mult)
                        mts.append(mt)
                    for wl in range(CW):
                        w = kc * CW + wl
                        pa = pa_ps.tile([FMAX, P], dt.float32, tag="pa", name="pa")[:F_in]
                        for kq in range(4):
                            a = ((kc * 4 + kq) * CW + wl) * cc
                            S = sp.tile([P, cc, P], dt.bfloat16, tag="S")
                            nc.vector.tensor_tensor(
                                out=S[:],
                                in0=edl[:, a:a + cc, None].broadcast_to([P, cc, P]),
                                in1=iota_b[:, None, :].broadcast_to([P, cc, P]),
                                op=OP.is_equal)
                            for j in range(cc):
                                nc.tensor.matmul(
                                    out=pa[:],
                                    lhsT=mts[kq][:, wl * cc + j, 0:F_in],
                                    rhs=S[:, j, :],
                                    start=(kq == 0 and j == 0),
                                    stop=(kq == 3 and j == cc - 1))
                        aggT = work.tile([FMAX, P], dt.bfloat16, tag="aggT", name="aggT")[:F_in]
                        nc.scalar.copy(out=aggT[:], in_=pa[:])
                        p2 = p2_ps.tile([P, F3], dt.float32, tag="p2", name="p2")[:, :F_out]
                        nc.tensor.matmul(out=p2[:], lhsT=aggT[:], rhs=waug[:F_in, :],
                                         start=True, stop=False)
                        nc.tensor.matmul(out=p2[:], lhsT=ones1[:], rhs=brow[:],
                                         start=False, stop=True)
                        if not last:
                            hp = work.tile([P, ELEM], dt.bfloat16, tag="hp",
                                           name="hp")
                            nc.scalar.activation(hp[:, 0:F_out], p2[:], AF.Relu)
                            nc.sync.dma_start(
                                out=shard_out[w * P:(w + 1) * P, :], in_=hp[:])
                        else:
                            h = work.tile([P, F3], dt.bfloat16, tag="h",
                                          name="h")[:, :F_out]
                            nc.scalar.activation(h[:], p2[:], AF.Relu)
                            Sg = sp.tile([P, P], dt.bfloat16, tag="Sg")
                            nc.vector.tensor_tensor(
                                out=Sg[:],
                                in0=gloc[:, w:w + 1].broadcast_to([P, P]),
                                in1=iota_b[:], op=OP.is_equal)
                            nc.tensor.matmul(out=pool_acc[:], lhsT=Sg[:], rhs=h[:],
                                             start=(w == 0), stop=(w == NW - 1))
                return do_chunk

            def seg_allgather(shard, table, sgi):
                nc.gpsimd.collective_compute(
                    "AllGather", mybir.AluOpType.bypass, replica_groups=rg,
                    ins=[shard[sgi * SEG:(sgi + 1) * SEG, :].opt()],
                    outs=[table[sgi * HSEG:(sgi + 1) * HSEG, :].opt()])

            for _rep in range(reps):
                l1 = layer(xtp_d, F0, F1, w1a, b1r, h1s)
                for kc in range(NCHUNK):
                    l1(kc)
                    if kc == NCHUNK // 2 - 1:
                        seg_allgather(h1s, h1t, 0)
                seg_allgather(h1s, h1t, 1)

                l2 = layer(h1t, F1, F2, w2a, b2r, h2s)
                for kc in range(NCHUNK):
                    l2(kc)
                    if kc == NCHUNK // 2 - 1:
                        seg_allgather(h2s, h2t, 0)
                seg_allgather(h2s, h2t, 1)

                l3 = layer(h2t, F2, F3, w3a, b3r, None)
                for kc in range(NCHUNK):
                    l3(kc)

            # ---- pooling partial -> AllReduce -> mean
            psb = work.tile([P, F3], dt.float32, tag="psb")
            nc.scalar.copy(out=psb[:], in_=pool_acc[:])
            nc.sync.dma_start(out=pool_pt[:], in_=psb[:])
            nc.gpsimd.collective_compute(
                "AllReduce", mybir.AluOpType.add, replica_groups=rg,
                ins=[pool_pt.opt()], outs=[pool_rd.opt()])
            poolr = work.tile([P, F3], dt.float32, tag="poolr")
            nc.sync.dma_start(out=poolr[:], in_=pool_rd[:])
            pooled = work.tile([P, F3], dt.bfloat16, tag="pooled")
            nc.scalar.activation(pooled[:], poolr[:], AF.Copy, scale=invc[:])

            # ---- head: z1 = relu(pooled @ fW1 + fb1); z2 = z1 @ fW2 + fb2
            ptA_ps = head_ps.tile([F3 // 2, P], dt.bfloat16, tag="pt")
            nc.tensor.transpose(out=ptA_ps[:], in_=pooled[:, :F3 // 2], identity=ident[:])
            ptA = work.tile([F3 // 2, P], dt.bfloat16, tag="ptA")
            nc.scalar.copy(out=ptA[:], in_=ptA_ps[:])
            ptB_ps = head_ps.tile([F3 // 2, P], dt.bfloat16, tag="pt")
            nc.tensor.transpose(out=ptB_ps[:], in_=pooled[:, F3 // 2:], identity=ident[:])
            ptB = work.tile([F3 // 2, P], dt.bfloat16, tag="ptB")
            nc.scalar.copy(out=ptB[:], in_=ptB_ps[:])

            z1_ps = head_ps.tile([HID, P], dt.float32, tag="z1")
            nc.tensor.matmul(out=z1_ps[:], lhsT=fw1a[:], rhs=ptA[:], start=True, stop=False)
            nc.tensor.matmul(out=z1_ps[:], lhsT=fw1b[:], rhs=ptB[:], start=False, stop=True)
            z1 = work.tile([HID, P], dt.bfloat16, tag="z1s")
            nc.scalar.activation(z1[:], z1_ps[:], AF.Relu, bias=fb1c[:])

            z2_ps = head_ps.tile([1, P], dt.float32, tag="z2")
            nc.tensor.matmul(out=z2_ps[:], lhsT=fw2[:], rhs=z1[:], start=True, stop=True)
            z2 = work.tile([1, P], dt.float32, tag="z2s")
            nc.scalar.activation(z2[:], z2_ps[:], AF.Copy, bias=float(fb2))
            # softmax over a width-1 axis == 1.0 for finite logits
            outs = work.tile([1, P], dt.float32, tag="outs")
            nc.vector.tensor_tensor(out=outs[:], in0=z2[:], in1=z2[:], op=OP.is_equal)
            nc.sync.dma_start(out=out_d[:], in_=outs[:])

    nc.compile()
    return nc


# ---------------------------------------------------------------- run

_CACHE = {}


def _get_nc(cc, fb2, reps=1):
    key = (cc, fb2, reps)
    if key not in _CACHE:
        _CACHE[key] = build_bass(cc, fb2, reps)
    return _CACHE[key]


def make_in_maps(inputs, cc):
    cores, xtp, wts, fb2 = build_host_data(inputs, cc)
    in_maps = [dict(xtp=xtp, **cores[c], **wts) for c in range(NCORES)]
    return in_maps, fb2


def kernel(**inputs):
    cc = derive_cc(inputs)
    in_maps, fb2 = make_in_maps(inputs, cc)
    nc = _get_nc(cc, fb2)
    from concourse.bass_utils import run_bass_kernel_spmd
    res = run_bass_kernel_spmd(nc, in_maps, core_ids=list(range(NCORES)))
    out = np.asarray(res.results[0]["out"]).reshape(P)[:G]
    return out.reshape(G, 1).astype(np.float32)


# revision 3
# speedup vs baseline: 1.0215x; 1.0215x over previous
"""GCN (3-layer GCNConv + mean-pool + MLP head) Trainium2 Bass kernel, 8 NeuronCores.

v2: bulk hardware gather via gpsimd.dma_gather (SWDGE descriptor gather,
~0.34ns/descriptor desc-gen) replaces the per-column indirect DMA of the
baseline (~15us/instruction software descriptor generation).

Strategy:
  - Destination nodes partitioned into 8 contiguous blocks (one per core),
    SHARD=12544 padded rows each; node tables live in DRAM padded to 128
    bf16 elements per row (256B, the dma_gather granularity).
  - Self-loops folded into the edge list as ordinary edges (weight 1/deg).
  - dma_gather indices are int16, so the 100352-row table is addressed in
    4 quarters of 25088 rows; edges are packed into 128-edge columns
    grouped by (chunk of 7 dst windows, src quarter, dst window) with a
    fixed CC columns per (window, quarter) group -> fully static layout,
    identical across cores (SPMD-safe).
  - One dma_gather per (chunk, quarter) fetches CW*CC columns of message
    rows; messages are scaled by the GCN edge norm and scattered into a
    per-window PSUM accumulator with one-hot matmuls, then transformed
    (aggregate-then-transform), relu'd, and written to the padded shard.
  - Layer boundary: two AllGather segments (49 windows each, aligned to
    chunk boundaries) so the seg-0 collective overlaps second-half compute
    and the next layer's quarter-0/1 gathers overlap the seg-1 collective.
  - Layer 3 output is mean-pooled per graph (one-hot matmul into a PSUM
    accumulator), AllReduced, and the tiny FC head runs replicated.
"""

import os
import sys

import numpy as np
import ml_dtypes

for _p in ("/opt/trn_rl_repo", "/root/.axon_site/_ro/trn_rl_repo"):
    if os.path.isdir(_p) and _p not in sys.path:
        sys.path.insert(0, _p)

bf16 = ml_dtypes.bfloat16
P = 128

N = 100000
G = 128
SHARD = 12544
NW = 98           # dst windows per core
CW = 7            # windows per chunk
NCHUNK = NW // CW
SEG = SHARD // 2  # AllGather segment rows (= 49 windows)
NPAD = 8 * SHARD  # 100352
QR = NPAD // 4    # quarter rows: 25088 (int16-addressable)
FS = (40, 40, 80, 160)
HID = 128
NCORES = 8
ELEM = 128        # padded table row elements (256B)
SCRATCH = 16384   # SWDGE descriptor scratch (ring capacity is runtime-fixed)
MAXCOLS = 8       # max 128-edge columns per dma_gather call (ring ~80 descs)


# ---------------------------------------------------------------- host prep

def _remap(v):
    c, r = v // SHARD, v % SHARD
    s = r // SEG
    return s * (NCORES * SEG) + c * SEG + (r - s * SEG)


def call_ranges(cc):
    """Gather-call column ranges: each (chunk, quarter) group of CW*cc
    columns split into sub-calls of <= MAXCOLS columns (SWDGE ring limit)."""
    ccall = CW * cc
    nsplit = -(-ccall // MAXCOLS)
    out = []
    for kc in range(NCHUNK):
        for kq in range(4):
            a = ((kc * 4 + kq) * CW) * cc
            for s in range(nsplit):
                lo = a + s * ccall // nsplit
                hi = a + (s + 1) * ccall // nsplit
                if hi > lo:
                    out.append((lo, hi))
    return out


def build_host_data(inp, cc):
    """Pack edges (incl. self-loops) into the fixed grid: group
    (chunk, quarter, window-in-chunk) owns `cc` 128-edge columns."""
    src = np.asarray(inp["edge_index"][0]).astype(np.int64).ravel()
    dst = np.asarray(inp["edge_index"][1]).astype(np.int64).ravel()
    batch = np.asarray(inp["batch"]).astype(np.int64).ravel()

    deg = (np.bincount(dst, minlength=N) + 1).astype(np.float32)
    dis = 1.0 / np.sqrt(deg)

    loop = np.arange(N, dtype=np.int64)
    srcA = np.concatenate([src, loop])
    dstA = np.concatenate([dst, loop])
    norm = (dis[srcA] * dis[dstA]).astype(np.float32)

    rsrc = _remap(srcA)
    q = rsrc // QR
    qi = (rsrc % QR).astype(np.int16)
    core = dstA // SHARD
    w = (dstA % SHARD) // P
    dloc = (dstA % SHARD) % P

    COLS = NW * 4 * cc  # group (w, q) -> columns gbase(w, q) + [0, cc)

    def gbase(w_, q_):
        kc, wl = w_ // CW, w_ % CW
        return ((kc * 4 + q_) * CW + wl) * cc

    order = np.lexsort((q, w, core))
    qA, qiA, coreA, wA, dlocA, normA = (
        a[order] for a in (q, qi, core, w, dloc, norm))

    cores = []
    for c in range(NCORES):
        lo, hi = np.searchsorted(coreA, [c, c + 1])
        cq, cqi, cw_, cdl, cnm = (
            a[lo:hi] for a in (qA, qiA, wA, dlocA, normA))
        gkey = cw_ * 4 + cq  # sorted (w asc, q asc) by the lexsort
        ukeys, counts = np.unique(gkey, return_counts=True)
        assert counts.max() <= cc * P, f"group overflow {counts.max()} > {cc * P}"
        estart = np.concatenate([[0], np.cumsum(counts)])
        jin = np.arange(len(cq)) - np.repeat(estart[:-1], counts)
        gb = np.array([gbase(k // 4, k % 4) for k in ukeys])
        col = np.repeat(gb, counts) + jin // P
        pp = jin % P

        eidx = np.zeros((P, COLS), np.int16)
        edl = np.full((P, COLS), -1.0, bf16)
        enrm = np.zeros((P, COLS), bf16)
        eidx[pp, col] = cqi
        edl[pp, col] = cdl.astype(bf16)
        enrm[pp, col] = cnm.astype(bf16)

        # wrapped idx per gather call (sub-ranges of a chunk-quarter group):
        # idx k of a call -> wrapped [partition k%16 (tiled x8), col k//16]
        widx = np.zeros((P, COLS * 8), np.int16)
        for a, b in call_ranges(cc):
            arr = eidx[:, a:b].T.ravel()
            wrap = arr.reshape(-1, 16).T
            widx[:, a * 8:b * 8] = np.tile(wrap, (8, 1))

        nid = np.arange(SHARD) + c * SHARD
        gl = np.where(nid < N, batch[np.minimum(nid, N - 1)], -1).astype(np.float32)
        gloc = np.ascontiguousarray(gl.reshape(NW, P).T).astype(bf16)

        cores.append(dict(eidxw=widx, edl=edl, enrm=enrm, gloc=gloc))

    xtp = np.zeros((NPAD, ELEM), bf16)
    xtp[_remap(np.arange(N)), :FS[0]] = np.asarray(inp["x"]).astype(bf16)

    cnt = np.bincount(batch, minlength=G).astype(np.float32)
    invc = np.zeros((P, 1), np.float32)
    invc[:G, 0] = 1.0 / np.maximum(cnt, 1.0)

    def a2(x, dt):
        return np.ascontiguousarray(np.asarray(x), dtype=dt)

    wts = dict(
        w1a=np.concatenate([a2(inp["W1"], bf16), a2(inp["b1"], bf16)[None]], 0),
        w2a=np.concatenate([a2(inp["W2"], bf16), a2(inp["b2"], bf16)[None]], 0),
        w3a=np.concatenate([a2(inp["W3"], bf16), a2(inp["b3"], bf16)[None]], 0),
        fw1=a2(inp["fW1"], bf16),
        fb1c=a2(inp["fb1"], np.float32).reshape(-1, 1),
        fw2=a2(inp["fW2"], bf16),
        invc=invc,
        iob=np.arange(P, dtype=np.float32)[None].repeat(P, 0).astype(bf16),
        idn=np.eye(P, dtype=bf16),
    )
    fb2 = float(np.asarray(inp["fb2"]).ravel()[0])
    return cores, xtp, wts, fb2


def derive_cc(inp):
    """cc = max columns any (dst-window, src-quarter) group needs."""
    src = np.asarray(inp["edge_index"][0]).astype(np.int64).ravel()
    dst = np.asarray(inp["edge_index"][1]).astype(np.int64).ravel()
    loop = np.arange(N, dtype=np.int64)
    srcA = np.concatenate([src, loop])
    dstA = np.concatenate([dst, loop])
    q = _remap(srcA) // QR
    gw = dstA // P  # global window id (SHARD % 128 == 0)
    cnt = np.bincount(gw * 4 + q, minlength=(NPAD // P) * 4)
    return max(1, int(-(-cnt.max() // P)))


# ---------------------------------------------------------------- bass build

def build_bass(cc, fb2, reps=1):
    import concourse.bacc as bacc
    import concourse.mybir as mybir
    import concourse.tile as tile

    dt = mybir.dt
    AF = mybir.ActivationFunctionType
    OP = mybir.AluOpType
    F0, F1, F2, F3 = FS
    FMAX = max(F0, F1, F2)
    COLS = NW * 4 * cc
    CCALL = CW * cc

    nc = bacc.Bacc("TRN2", target_bir_lowering=False, debug=False,
                   enable_asserts=False, num_devices=NCORES,
                   num_swdge_queues=4, dynamic_dma_scratch_size=SCRATCH)

    grp_calls = {}
    for lo, hi in call_ranges(cc):
        g = lo // (CW * cc)
        grp_calls.setdefault((g // 4, g % 4), []).append((lo, hi))

    xtp_d = nc.dram_tensor("xtp", [NPAD, ELEM], dt.bfloat16, kind="ExternalInput")
    widx_d = nc.dram_tensor("eidxw", [P, COLS * 8], dt.int16, kind="ExternalInput")
    edl_d = nc.dram_tensor("edl", [P, COLS], dt.bfloat16, kind="ExternalInput")
    enrm_d = nc.dram_tensor("enrm", [P, COLS], dt.bfloat16, kind="ExternalInput")
    gloc_d = nc.dram_tensor("gloc", [P, NW], dt.bfloat16, kind="ExternalInput")
    w1a_d = nc.dram_tensor("w1a", [F0 + 1, F1], dt.bfloat16, kind="ExternalInput")
    w2a_d = nc.dram_tensor("w2a", [F1 + 1, F2], dt.bfloat16, kind="ExternalInput")
    w3a_d = nc.dram_tensor("w3a", [F2 + 1, F3], dt.bfloat16, kind="ExternalInput")
    fw1_d = nc.dram_tensor("fw1", [F3, HID], dt.bfloat16, kind="ExternalInput")
    fb1_d = nc.dram_tensor("fb1c", [HID, 1], dt.float32, kind="ExternalInput")
    fw2_d = nc.dram_tensor("fw2", [HID, 1], dt.bfloat16, kind="ExternalInput")
    invc_d = nc.dram_tensor("invc", [P, 1], dt.float32, kind="ExternalInput")
    iob_d = nc.dram_tensor("iob", [P, P], dt.bfloat16, kind="ExternalInput")
    idn_d = nc.dram_tensor("idn", [P, P], dt.bfloat16, kind="ExternalInput")
    out_d = nc.dram_tensor("out", [1, P], dt.float32, kind="ExternalOutput")

    rg = [list(range(NCORES))]
    HSEG = NCORES * SEG

    with tile.TileContext(nc) as tc:
        with (
            tc.tile_pool(name="res", bufs=1) as res,
            tc.tile_pool(name="msgs", bufs=8) as msgsp,
            tc.tile_pool(name="sp", bufs=3) as sp,
            tc.tile_pool(name="work", bufs=3) as work,
            tc.tile_pool(name="pa_ps", bufs=2, space="PSUM") as pa_ps,
            tc.tile_pool(name="p2_ps", bufs=2, space="PSUM") as p2_ps,
            tc.tile_pool(name="pool_ps", bufs=1, space="PSUM") as pool_ps,
            tc.tile_pool(name="head_ps", bufs=1, space="PSUM") as head_ps,
            tc.tile_pool(name="dram", bufs=1, space="DRAM") as dram,
        ):
            # ---- persistent SBUF state
            widx = res.tile([P, COLS * 8], dt.int16)
            edl = res.tile([P, COLS], dt.bfloat16)
            enrm = res.tile([P, COLS], dt.bfloat16)
            gloc = res.tile([P, NW], dt.bfloat16)
            w1a = res.tile([F0 + 1, F1], dt.bfloat16)
            w2a = res.tile([F1 + 1, F2], dt.bfloat16)
            w3a = res.tile([F2 + 1, F3], dt.bfloat16)
            fw1a = res.tile([F3 // 2, HID], dt.bfloat16)
            fw1b = res.tile([F3 // 2, HID], dt.bfloat16)
            fb1c = res.tile([HID, 1], dt.float32)
            fw2 = res.tile([HID, 1], dt.bfloat16)
            invc = res.tile([P, 1], dt.float32)
            b1r = res.tile([1, F1], dt.bfloat16)
            b2r = res.tile([1, F2], dt.bfloat16)
            b3r = res.tile([1, F3], dt.bfloat16)
            iota_b = res.tile([P, P], dt.bfloat16)
            ident = res.tile([P, P], dt.bfloat16)
            ones1 = res.tile([1, P], dt.bfloat16)
            for sb, dr in ((widx, widx_d), (edl, edl_d), (enrm, enrm_d),
                           (gloc, gloc_d), (w1a, w1a_d), (w2a, w2a_d),
                           (w3a, w3a_d), (fb1c, fb1_d), (fw2, fw2_d),
                           (invc, invc_d), (iota_b, iob_d), (ident, idn_d)):
                nc.sync.dma_start(out=sb[:], in_=dr[:])
            nc.sync.dma_start(out=b1r[:], in_=w1a_d[F0:F0 + 1, :])
            nc.sync.dma_start(out=b2r[:], in_=w2a_d[F1:F1 + 1, :])
            nc.sync.dma_start(out=b3r[:], in_=w3a_d[F2:F2 + 1, :])
            nc.sync.dma_start(out=fw1a[:], in_=fw1_d[0:F3 // 2, :])
            nc.sync.dma_start(out=fw1b[:], in_=fw1_d[F3 // 2:, :])
            nc.vector.memset(ones1[:], 1.0)

            # ---- DRAM tables (padded 256B rows) / shard bounce buffers
            h1s = dram.tile([SHARD, ELEM], dt.bfloat16)
            h2s = dram.tile([SHARD, ELEM], dt.bfloat16)
            h1t = dram.tile([NPAD, ELEM], dt.bfloat16)
            h2t = dram.tile([NPAD, ELEM], dt.bfloat16)
            pool_pt = dram.tile([P, F3], dt.float32)
            pool_rd = dram.tile([P, F3], dt.float32)

            pool_acc = pool_ps.tile([P, F3], dt.float32)
            qrot = [0]

            def layer(tbl, F_in, F_out, waug, brow, shard_out):
                last = F_in == F2
                mts = {}

                def gather(kc, kq):
                    # emit the (chunk, quarter) gather once; prefetchable so
                    # seg0-only quarters can run ahead of a pending seg1
                    # AllGather (Pool SEQ is in-order).
                    if (kc, kq) in mts:
                        return
                    a = ((kc * 4 + kq) * CW) * cc
                    mt = msgsp.tile([P, CCALL, ELEM], dt.bfloat16, tag="mt",
                                    name=f"mt{kc}_{kq}")
                    for lo, hi in grp_calls[kc, kq]:
                        w_ = hi - lo
                        nc.gpsimd.dma_gather(
                            mt[:, lo - a:hi - a, :],
                            tbl[kq * QR:(kq + 1) * QR, :],
                            widx[:, lo * 8:hi * 8],
                            w_ * P, w_ * P, ELEM,
                            queue_num=qrot[0] % 4)
                        qrot[0] += 1
                    nc.vector.tensor_tensor(
                        out=mt[:, :, 0:F_in], in0=mt[:, :, 0:F_in],
                        in1=enrm[:, a:a + CCALL, None].broadcast_to(
                            [P, CCALL, F_in]),
                        op=OP.mult)
                    mts[(kc, kq)] = mt

                def do_chunk(kc):
                    for kq in range(4):
                        gather(kc, kq)
                    for wl in range(CW):
                        w = kc * CW + wl
                        pa = pa_ps.tile([FMAX, P], dt.float32, tag="pa", name="pa")[:F_in]
                        for kq in range(4):
                            a = ((kc * 4 + kq) * CW + wl) * cc
                            S = sp.tile([P, cc, P], dt.bfloat16, tag="S")
                            nc.vector.tensor_tensor(
                                out=S[:],
                                in0=edl[:, a:a + cc, None].broadcast_to([P, cc, P]),
                                in1=iota_b[:, None, :].broadcast_to([P, cc, P]),
                                op=OP.is_equal)
                            for j in range(cc):
                                nc.tensor.matmul(
                                    out=pa[:],
                                    lhsT=mts[(kc, kq)][:, wl * cc + j, 0:F_in],
                                    rhs=S[:, j, :],
                                    start=(kq == 0 and j == 0),
                                    stop=(kq == 3 and j == cc - 1))
                        aggT = work.tile([FMAX, P], dt.bfloat16, tag="aggT", name="aggT")[:F_in]
                        nc.scalar.copy(out=aggT[:], in_=pa[:])
                        p2 = p2_ps.tile([P, F3], dt.float32, tag="p2", name="p2")[:, :F_out]
                        nc.tensor.matmul(out=p2[:], lhsT=aggT[:], rhs=waug[:F_in, :],
                                         start=True, stop=False)
                        nc.tensor.matmul(out=p2[:], lhsT=ones1[:], rhs=brow[:],
                                         start=False, stop=True)
                        if not last:
                            hp = work.tile([P, ELEM], dt.bfloat16, tag="hp",
                                           name="hp")
                            nc.scalar.activation(hp[:, 0:F_out], p2[:], AF.Relu)
                            nc.sync.dma_start(
                                out=shard_out[w * P:(w + 1) * P, :], in_=hp[:])
                        else:
                            h = work.tile([P, F3], dt.bfloat16, tag="h",
                                          name="h")[:, :F_out]
                            nc.scalar.activation(h[:], p2[:], AF.Relu)
                            Sg = sp.tile([P, P], dt.bfloat16, tag="Sg")
                            nc.vector.tensor_tensor(
                                out=Sg[:],
                                in0=gloc[:, w:w + 1].broadcast_to([P, P]),
                                in1=iota_b[:], op=OP.is_equal)
                            nc.tensor.matmul(out=pool_acc[:], lhsT=Sg[:], rhs=h[:],
                                             start=(w == 0), stop=(w == NW - 1))
                return gather, do_chunk

            def seg_allgather(shard, table, sgi):
                nc.gpsimd.collective_compute(
                    "AllGather", mybir.AluOpType.bypass, replica_groups=rg,
                    ins=[shard[sgi * SEG:(sgi + 1) * SEG, :].opt()],
                    outs=[table[sgi * HSEG:(sgi + 1) * HSEG, :].opt()])

            for _rep in range(reps):
                def prefetch(g):
                    # first two chunks' quarter-0/1 gathers read only the
                    # seg-0 half of the table; emitting them before any
                    # quarter-2/3 gather hides the seg-1 AllGather latency
                    for kc in (0, 1):
                        for kq in (0, 1):
                            g(kc, kq)

                l1g, l1 = layer(xtp_d, F0, F1, w1a, b1r, h1s)
                prefetch(l1g)
                for kc in range(NCHUNK):
                    l1(kc)
                    if kc == NCHUNK // 2 - 1:
                        seg_allgather(h1s, h1t, 0)
                seg_allgather(h1s, h1t, 1)

                l2g, l2 = layer(h1t, F1, F2, w2a, b2r, h2s)
                prefetch(l2g)
                for kc in range(NCHUNK):
                    l2(kc)
                    if kc == NCHUNK // 2 - 1:
                        seg_allgather(h2s, h2t, 0)
                seg_allgather(h2s, h2t, 1)

                l3g, l3 = layer(h2t, F2, F3, w3a, b3r, None)
                prefetch(l3g)
                for kc in range(NCHUNK):
                    l3(kc)

            # ---- pooling partial -> AllReduce -> mean
            psb = work.tile([P, F3], dt.float32, tag="psb")
            nc.scalar.copy(out=psb[:], in_=pool_acc[:])
            nc.sync.dma_start(out=pool_pt[:], in_=psb[:])
            nc.gpsimd.collective_compute(
                "AllReduce", mybir.AluOpType.add, replica_groups=rg,
                ins=[pool_pt.opt()], outs=[pool_rd.opt()])
            poolr = work.tile([P, F3], dt.float32, tag="poolr")
            nc.sync.dma_start(out=poolr[:], in_=pool_rd[:])
            pooled = work.tile([P, F3], dt.bfloat16, tag="pooled")
            nc.scalar.activation(pooled[:], poolr[:], AF.Copy, scale=invc[:])

            # ---- head: z1 = relu(pooled @ fW1 + fb1); z2 = z1 @ fW2 + fb2
            ptA_ps = head_ps.tile([F3 // 2, P], dt.bfloat16, tag="pt")
            nc.tensor.transpose(out=ptA_ps[:], in_=pooled[:, :F3 // 2], identity=ident[:])
            ptA = work.tile([F3 // 2, P], dt.bfloat16, tag="ptA")
            nc.scalar.copy(out=ptA[:], in_=ptA_ps[:])
            ptB_ps = head_ps.tile([F3 // 2, P], dt.bfloat16, tag="pt")
            nc.tensor.transpose(out=ptB_ps[:], in_=pooled[:, F3 // 2:], identity=ident[:])
            ptB = work.tile([F3 // 2, P], dt.bfloat16, tag="ptB")
            nc.scalar.copy(out=ptB[:], in_=ptB_ps[:])

            z1_ps = head_ps.tile([HID, P], dt.float32, tag="z1")
            nc.tensor.matmul(out=z1_ps[:], lhsT=fw1a[:], rhs=ptA[:], start=True, stop=False)
            nc.tensor.matmul(out=z1_ps[:], lhsT=fw1b[:], rhs=ptB[:], start=False, stop=True)
            z1 = work.tile([HID, P], dt.bfloat16, tag="z1s")
            nc.scalar.activation(z1[:], z1_ps[:], AF.Relu, bias=fb1c[:])

            z2_ps = head_ps.tile([1, P], dt.float32, tag="z2")
            nc.tensor.matmul(out=z2_ps[:], lhsT=fw2[:], rhs=z1[:], start=True, stop=True)
            z2 = work.tile([1, P], dt.float32, tag="z2s")
            nc.scalar.activation(z2[:], z2_ps[:], AF.Copy, bias=float(fb2))
            # softmax over a width-1 axis == 1.0 for finite logits
            outs = work.tile([1, P], dt.float32, tag="outs")
            nc.vector.tensor_tensor(out=outs[:], in0=z2[:], in1=z2[:], op=OP.is_equal)
            nc.sync.dma_start(out=out_d[:], in_=outs[:])

    nc.compile()
    return nc


# ---------------------------------------------------------------- run

_CACHE = {}


def _get_nc(cc, fb2, reps=1):
    key = (cc, fb2, reps)
    if key not in _CACHE:
        _CACHE[key] = build_bass(cc, fb2, reps)
    return _CACHE[key]


def make_in_maps(inputs, cc):
    cores, xtp, wts, fb2 = build_host_data(inputs, cc)
    in_maps = [dict(xtp=xtp, **cores[c], **wts) for c in range(NCORES)]
    return in_maps, fb2


def kernel(**inputs):
    cc = derive_cc(inputs)
    in_maps, fb2 = make_in_maps(inputs, cc)
    nc = _get_nc(cc, fb2)
    from concourse.bass_utils import run_bass_kernel_spmd
    res = run_bass_kernel_spmd(nc, in_maps, core_ids=list(range(NCORES)))
    out = np.asarray(res.results[0]["out"]).reshape(P)[:G]
    return out.reshape(G, 1).astype(np.float32)
